# revision 40
# baseline (speedup 1.0000x reference)
"""Multi-head self-attention 2D (dense transformer) Bass kernel for Trainium2.

Problem: x [4, 512, 48, 48] fp32; qkv_w [1536, 512]; proj_w [512, 512].
  qkv 1x1-conv -> per-head attention (8 heads, head_dim 64) over N=2304
  spatial positions -> output projection.

Sharding (8 cores): core i handles batch b = i//2 and query half i%2
  (nq = 1152 queries). The host ROLLS each core's x so its query columns
  are always cols 0:1152 (attention is permutation-invariant over key
  positions, and V is computed from the same rolled x, so this is exact).
  Per-core outputs are disjoint slices of the full output -- no
  collectives, gather on host.

Per-core pipeline (matmul operands fp16, PSUM fp32). The ScalarE exp
stream (~21M elements/core, ~165us busy) and the PE matmul stream
(~188us busy) are the two walls; everything is emission-ordered so both
start as early as possible and neither starves the other:
  1. Input DMAs ordered strictly by need ((x tile, k-weight block)
     pairs first); exp activation table prewarmed during the DMA window.
  2. k/q for head-pair 0 first, then pair-0 attention begins; the V
     1x1-conv is woven into pair-0's first query chunk (v[m] emitted
     just before the AV matmul that consumes it), and k/q for pair t+1
     are spread in small groups through pair t's chunks (never exceeding
     the ~2-exp score-PSUM backlog that keeps ScalarE fed).
  3. attention per head-pair with transposed scores S^T = k_h^T q_h
     (row-packed PE pairs at tile_position (0,0)/(64,0)), wide exp
     [128, 1024] PSUM->SBUF on ScalarE (scale=1/8 folded in), AV
     matmuls accumulate in PSUM with a 65th ones-row of V producing
     softmax denominators for free.  Softmax max-subtraction is
     skipped: scores*scale ~ N(0,1) so exp stays in range.
  4. normalization per query-chunk, woven into the FOLLOWING chunk so
     its PE broadcast matmul (gated on a DVE reciprocal chain) never
     blocks the PE queue: reciprocal on partitions {0,32}, K=33
     selector matmul broadcast, one DVE multiply.
  5. projection y = Wp @ out accumulated across head-pairs in PSUM,
     woven piecewise into pair 3 (the small remainder chunk runs last
     so the post-last-exp tail is short), evacuated on DVE, DMAed out
     fp16 per piece (host casts back to fp32).
"""

import numpy as np

B = 4
C = 512
HH = 48
WW = 48
N = HH * WW          # 2304
NQ = N // 2          # 1152 queries per core
HEADS = 8
D = C // HEADS       # 64
SCALE = float(D) ** -0.5
NCORES = 8

_CACHE: dict = {}


def _build_module(loop_n=None):
    import concourse.mybir as mybir
    import concourse.tile as tile
    from concourse import bacc

    FP16 = mybir.dt.float16
    FP32 = mybir.dt.float32
    AF = mybir.ActivationFunctionType

    nc = bacc.Bacc("TRN2", target_bir_lowering=False, debug=False)
    xk = nc.dram_tensor("xk", [C, N], FP16, kind="ExternalInput")
    wqkv = nc.dram_tensor("wqkv", [C, 3 * C], FP16, kind="ExternalInput")
    wproj = nc.dram_tensor("wproj", [C, C], FP16, kind="ExternalInput")
    y = nc.dram_tensor("y", [C, NQ], FP16, kind="ExternalOutput")

    CT = C // 128     # 4 channel tiles
    MT = N // 128     # 18 key tiles
    NQ32 = [(0, 512), (512, 512), (1024, 128)]
    KCH = [(0, 512), (512, 512), (1024, 512), (1536, 512), (2048, 256)]

    with tile.TileContext(nc) as tc:
        with (
            tc.tile_pool(name="consts", bufs=1) as cpool,
            tc.tile_pool(name="wts", bufs=1) as wpool,
            tc.tile_pool(name="xs", bufs=1) as xpool,
            tc.tile_pool(name="qk", bufs=1) as qkpool,
            tc.tile_pool(name="keep", bufs=1) as keep,
            tc.tile_pool(name="ph1", bufs=2, space="PSUM") as ph1,
            tc.tile_pool(name="sps", bufs=2, space="PSUM") as sps,
            tc.tile_pool(name="avps", bufs=1, space="PSUM") as avps,
            tc.tile_pool(name="esb", bufs=6) as epool,
            tc.tile_pool(name="p2sb", bufs=2) as p2sb,
        ):
            # bce: K=33 selector for broadcasting the two per-half
            # reciprocal rows (on partitions 0 and 32 -- engine accesses
            # must start at multiples of 32) across partitions 0:64/64:128.
            bce = cpool.tile([33, 128], FP16, name="bce", tag="bce")
            nc.vector.memset(bce[:], 0.0)
            nc.vector.memset(bce[0:1, 0:64], 1.0)
            nc.vector.memset(bce[32:33, 64:128], 1.0)
            # prewarm the exp activation table (~2.7us load) during the
            # input DMA window so the first real exp doesn't pay it.
            warm = cpool.tile([1, 16], FP16, name="warm", tag="warm")
            nc.vector.memset(warm[:], 0.0)
            nc.scalar.activation(warm[:, 0:8], warm[:, 8:16], AF.Exp)

            # Input DMAs split across BOTH HWDGE queues (SP + Activation)
            # to halve the serialized startup latency; the small k/q weight
            # blocks go first so pair-0 scores can start earliest.
            wt = [wpool.tile([128, 3 * C], FP16, name=f"w{t}", tag=f"w{t}") for t in range(CT)]
            wp = [wpool.tile([128, C], FP16, name=f"wp{t}", tag=f"wp{t}") for t in range(CT)]
            xf = [xpool.tile([128, N], FP16, name=f"x{t}", tag=f"x{t}") for t in range(CT)]

            def dma_w(eng, t, blk):
                eng.dma_start(
                    wt[t][:, blk * C : (blk + 1) * C],
                    wqkv.ap()[128 * t : 128 * (t + 1), blk * C : (blk + 1) * C],
                )

            # The DMA path serializes per-transfer, so keep the COUNT low
            # and order strictly by need: (x tile, k-weight block) pairs so
            # the k accumulation matmuls start as each pair lands, then q,
            # v, proj weight blocks.
            for t in range(CT):
                nc.sync.dma_start(xf[t][:], xk.ap()[128 * t : 128 * (t + 1), :])
                dma_w(nc.scalar, t, 1)   # k blocks
            for t in range(CT):
                dma_w(nc.scalar, t, 0)   # q blocks
            for t in range(CT):
                dma_w(nc.sync, t, 2)     # v blocks
            for t in range(CT):
                nc.scalar.dma_start(wp[t][:], wproj.ap()[128 * t : 128 * (t + 1), :])

            qsb = [qkpool.tile([128, NQ], FP16, name=f"q{t}", tag=f"q{t}") for t in range(CT)]
            ksb = [qkpool.tile([128, N], FP16, name=f"k{t}", tag=f"k{t}") for t in range(CT)]
            vsb = [qkpool.tile([128, 520], FP16, name=f"v{m}", tag=f"v{m}") for m in range(MT)]

            avsb = [keep.tile([128, NQ], FP16, name=f"av{t}", tag=f"av{t}") for t in range(CT)]
            # denominator rows live on partitions 0 (half A) and 32 (half
            # B); rows 1..31 are set to 1.0 once so their reciprocal stays
            # finite (the bce selector zeroes them in the broadcast).
            cssb = [keep.tile([33, NQ], FP32, name=f"cs{t}", tag=f"cs{t}") for t in range(CT)]
            for t in range(CT):
                nc.vector.memset(cssb[t][:], 1.0)
            oa = [keep.tile([128, NQ], FP16, name=f"oa{t}", tag=f"oa{t}") for t in range(CT)]

            def split_gemm(dest, wcol, c0, cl):
                ph = ph1.tile([128, 512], FP32, name="ph", tag="ph")
                for kt in range(CT):
                    nc.tensor.matmul(
                        ph[:, 0:cl],
                        lhsT=wt[kt][:, wcol : wcol + 128],
                        rhs=xf[kt][:, c0 : c0 + cl],
                        start=(kt == 0), stop=(kt == CT - 1),
                    )
                nc.vector.tensor_copy(dest, ph[:, 0:cl])

            def emit_q_group(t, c0, cl):
                split_gemm(qsb[t][:, c0 : c0 + cl], 128 * t, c0, cl)

            def emit_k_group(t, c0, cl):
                split_gemm(ksb[t][:, c0 : c0 + cl], C + 128 * t, c0, cl)

            def emit_q(t):
                for c0, cl in NQ32:
                    emit_q_group(t, c0, cl)

            def emit_k(t):
                for c0, cl in KCH:
                    emit_k_group(t, c0, cl)

            def emit_v(m):
                # vT[n, cv] = sum_c x[c, n] * WvT[c, cv]; layout per key
                # tile: [128 keys, 8 heads x (64 dims + ones col)].
                ph = ph1.tile([128, 512], FP32, name="ph", tag="ph")
                for kt in range(CT):
                    nc.tensor.matmul(
                        ph[:, 0:512],
                        lhsT=xf[kt][:, 128 * m : 128 * (m + 1)],
                        rhs=wt[kt][:, 2 * C : 3 * C],
                        start=(kt == 0), stop=(kt == CT - 1),
                    )
                vv = vsb[m][:].rearrange("p (h w) -> p h w", h=8)
                nc.vector.memset(vv[:, :, 64:65], 1.0)
                nc.vector.tensor_copy(
                    vv[:, :, 0:64], ph[:].rearrange("p (h w) -> p h w", h=8)
                )

            def chunk(t, c, weave_v=False, weave=None):
                # main nq chunk of 512: scores + exp + AV over all key
                # tiles. `weave` maps m -> list of thunks emitted between
                # the exp and the AV matmuls (small PE detours placed so
                # the ScalarE exp backlog is never exceeded).
                nq0 = 512 * c
                kA = ksb[t][0:64, :]
                kB = ksb[t][64:128, :]
                qA = qsb[t][0:64, :]
                qB = qsb[t][64:128, :]
                avA = avps.tile([65, 512], FP32, name="avA", tag="avA")
                avB = avps.tile([65, 512], FP32, name="avB", tag="avB")
                for m in range(MT):
                    ms = slice(128 * m, 128 * (m + 1))
                    sp = sps.tile([128, 1024], FP32, name="s", tag="s")
                    nc.tensor.matmul(
                        sp[:, 0:512], lhsT=kA[:, ms],
                        rhs=qA[:, nq0 : nq0 + 512],
                        start=True, stop=True, tile_position=(0, 0),
                    )
                    nc.tensor.matmul(
                        sp[:, 512:1024], lhsT=kB[:, ms],
                        rhs=qB[:, nq0 : nq0 + 512],
                        start=True, stop=True, tile_position=(64, 0),
                    )
                    es = epool.tile([128, 1024], FP16, name="es", tag="es")
                    nc.scalar.activation(es[:], sp[:], AF.Exp, scale=SCALE)
                    if weave_v:
                        emit_v(m)
                    if weave and m in weave:
                        for fn in weave[m]:
                            fn()
                    nc.tensor.matmul(
                        avA[:], lhsT=vsb[m][:, 130 * t : 130 * t + 65],
                        rhs=es[:, 0:512],
                        start=(m == 0), stop=(m == MT - 1),
                    )
                    nc.tensor.matmul(
                        avB[:], lhsT=vsb[m][:, 130 * t + 65 : 130 * t + 130],
                        rhs=es[:, 512:1024],
                        start=(m == 0), stop=(m == MT - 1),
                    )
                nc.vector.tensor_copy(avsb[t][0:64, nq0 : nq0 + 512], avA[0:64, :])
                nc.vector.tensor_copy(avsb[t][64:128, nq0 : nq0 + 512], avB[0:64, :])
                nc.vector.tensor_copy(cssb[t][0:1, nq0 : nq0 + 512], avA[64:65, :])
                nc.vector.tensor_copy(cssb[t][32:33, nq0 : nq0 + 512], avB[64:65, :])

            def rem(t, weave=None):
                # remainder nq chunk (128 queries), exp batched over
                # groups of 4 key tiles: bank0 = A rems, bank1 = B rems
                kA = ksb[t][0:64, :]
                kB = ksb[t][64:128, :]
                qA = qsb[t][0:64, :]
                qB = qsb[t][64:128, :]
                avrA = avps.tile([65, 128], FP32, name="avrA", tag="avA")
                avrB = avps.tile([65, 128], FP32, name="avrB", tag="avB")
                for gi, g0 in enumerate(range(0, MT, 4)):
                    gm = min(4, MT - g0)
                    sr = sps.tile([128, 1024], FP32, name="sr", tag="s")
                    for j in range(gm):
                        ms = slice(128 * (g0 + j), 128 * (g0 + j + 1))
                        nc.tensor.matmul(
                            sr[:, 128 * j : 128 * j + 128], lhsT=kA[:, ms],
                            rhs=qA[:, 1024:1152],
                            start=True, stop=True, tile_position=(0, 0),
                        )
                        nc.tensor.matmul(
                            sr[:, 512 + 128 * j : 512 + 128 * j + 128], lhsT=kB[:, ms],
                            rhs=qB[:, 1024:1152],
                            start=True, stop=True, tile_position=(64, 0),
                        )
                    er = epool.tile([128, 1024], FP16, name="er", tag="er", bufs=2)
                    if gm == 4:
                        nc.scalar.activation(er[:], sr[:], AF.Exp, scale=SCALE)
                    else:
                        nc.scalar.activation(
                            er[:, 0 : 128 * gm], sr[:, 0 : 128 * gm],
                            AF.Exp, scale=SCALE,
                        )
                        nc.scalar.activation(
                            er[:, 512 : 512 + 128 * gm], sr[:, 512 : 512 + 128 * gm],
                            AF.Exp, scale=SCALE,
                        )
                    if weave and gi in weave:
                        for fn in weave[gi]:
                            fn()
                    for j in range(gm):
                        m = g0 + j
                        nc.tensor.matmul(
                            avrA[:], lhsT=vsb[m][:, 130 * t : 130 * t + 65],
                            rhs=er[:, 128 * j : 128 * j + 128],
                            start=(m == 0), stop=(m == MT - 1),
                        )
                        nc.tensor.matmul(
                            avrB[:], lhsT=vsb[m][:, 130 * t + 65 : 130 * t + 130],
                            rhs=er[:, 512 + 128 * j : 512 + 128 * j + 128],
                            start=(m == 0), stop=(m == MT - 1),
                        )
                nc.vector.tensor_copy(avsb[t][0:64, 1024:1152], avrA[0:64, :])
                nc.vector.tensor_copy(avsb[t][64:128, 1024:1152], avrB[0:64, :])
                nc.vector.tensor_copy(cssb[t][0:1, 1024:1152], avrA[64:65, :])
                nc.vector.tensor_copy(cssb[t][32:33, 1024:1152], avrB[64:65, :])

            def normalize(t, c0, cl):
                rec = p2sb.tile([33, 512], FP16, name="rec", tag="rec")
                with nc.allow_low_precision(reason="softmax recip fp16"):
                    nc.vector.reciprocal(rec[:, 0:cl], cssb[t][:, c0 : c0 + cl])
                bc = ph1.tile([128, 512], FP32, name="ph", tag="ph")
                nc.tensor.matmul(
                    bc[:, 0:cl], lhsT=bce[:],
                    rhs=rec[:, 0:cl],
                    start=True, stop=True,
                )
                nc.vector.tensor_mul(
                    oa[t][:, c0 : c0 + cl],
                    avsb[t][:, c0 : c0 + cl],
                    bc[:, 0:cl],
                )

            def proj_piece(c0, cl, ct):
                # y[ct] = sum_t WpT[t-block, ct-block].T @ oa[t], K-split
                # row-packed partials accumulated in PSUM; DVE-merge and
                # DMA out per piece, output queues alternating SP/ACT.
                py = ph1.tile([128, 512], FP32, name="ph", tag="ph")
                for t in range(CT):
                    nc.tensor.matmul(
                        py[:, 0:cl],
                        lhsT=wp[t][:, 128 * ct : 128 * (ct + 1)],
                        rhs=oa[t][:, c0 : c0 + cl],
                        start=(t == 0), stop=(t == CT - 1),
                    )
                yo = p2sb.tile([128, 512], FP16, name="yo", tag="yo")
                nc.vector.tensor_copy(yo[:, 0:cl], py[:, 0:cl])
                nc.sync.dma_start(
                    y.ap()[128 * ct : 128 * (ct + 1), c0 : c0 + cl],
                    yo[:, 0:cl],
                )

            def _run_iter():
                emit_k(0)
                emit_q(0)
                chunk(0, 0, weave_v=True)
                for t in range(CT):
                    # normalizations are woven into the FOLLOWING chunk so
                    # their PE broadcast matmul (gated on a DVE reciprocal
                    # chain) never sits at the head of the PE queue.
                    if t + 1 < CT:
                        # spread next pair's k/q groups through the chunks
                        # so the exp stream never starves on a PE detour;
                        # pair 0's chunk 0 carries the v weave, later pairs'
                        # chunk 0 takes the first k groups instead.
                        kq = {
                            1: [lambda t=t: normalize(t, 0, 512)],
                            2: [lambda t=t: emit_k_group(t + 1, *KCH[0])],
                            5: [lambda t=t: emit_k_group(t + 1, *KCH[1])],
                            8: [lambda t=t: emit_k_group(t + 1, *KCH[2])],
                            11: [lambda t=t: emit_k_group(t + 1, *KCH[3])],
                            14: [lambda t=t: emit_k_group(t + 1, *KCH[4])],
                            16: [lambda t=t: emit_q_group(t + 1, *NQ32[0])],
                            17: [lambda t=t: emit_q_group(t + 1, *NQ32[1])],
                        } if t == 0 else {
                            1: [lambda t=t: normalize(t, 0, 512)],
                            4: [lambda t=t: emit_k_group(t + 1, *KCH[3])],
                            8: [lambda t=t: emit_k_group(t + 1, *KCH[4])],
                            12: [lambda t=t: emit_q_group(t + 1, *NQ32[0])],
                            15: [lambda t=t: emit_q_group(t + 1, *NQ32[1])],
                        }
                        chunk(t, 1, weave=kq)
                        rem(t, weave={
                            0: [lambda t=t: emit_q_group(t + 1, *NQ32[2])],
                            1: [lambda t=t: normalize(t, 512, 512)],
                        })
                        nxt = {2: [lambda t=t: normalize(t, 1024, 128)]}
                        if t + 2 < CT:
                            # first k groups of pair t+2 ride in chunk 0 of
                            # pair t+1 (it has spare PE slack)
                            nxt[6] = [lambda t=t: emit_k_group(t + 2, *KCH[0])]
                            nxt[10] = [lambda t=t: emit_k_group(t + 2, *KCH[1])]
                            nxt[14] = [lambda t=t: emit_k_group(t + 2, *KCH[2])]
                        chunk(t + 1, 0, weave=nxt)
                    else:
                        # last pair: the small remainder chunk goes LAST so
                        # the post-last-exp tail is short; projections for
                        # cols 0:512 weave into chunk 1 and for 512:1024
                        # into the remainder.
                        pj = {
                            1: [lambda t=t: normalize(t, 0, 512)],
                            4: [lambda: proj_piece(0, 512, 0)],
                            7: [lambda: proj_piece(0, 512, 1)],
                            10: [lambda: proj_piece(0, 512, 2)],
                            13: [lambda: proj_piece(0, 512, 3)],
                        }
                        chunk(t, 1, weave=pj)
                        rem(t, weave={
                            0: [lambda t=t: normalize(t, 512, 512)],
                            2: [lambda: proj_piece(512, 512, 0)],
                            3: [lambda: proj_piece(512, 512, 1)],
                            4: [lambda: proj_piece(512, 512, 2)],
                        })
                        proj_piece(512, 512, 3)
                        normalize(t, 1024, 128)
                        # remainder projection: all four ct blocks packed in
                        # one PSUM tile -> one copy -> one rearranged DMA.
                        py = ph1.tile([128, 512], FP32, name="ph", tag="ph")
                        for ct in range(CT):
                            for tt in range(CT):
                                nc.tensor.matmul(
                                    py[:, 128 * ct : 128 * (ct + 1)],
                                    lhsT=wp[tt][:, 128 * ct : 128 * (ct + 1)],
                                    rhs=oa[tt][:, 1024:1152],
                                    start=(tt == 0), stop=(tt == CT - 1),
                                )
                        yo = p2sb.tile([128, 512], FP16, name="yo", tag="yo")
                        nc.vector.tensor_copy(yo[:], py[:])
                        nc.sync.dma_start(
                            y.ap()[:, 1024:1152].rearrange("(c p) w -> p c w", c=4),
                            yo[:].rearrange("p (c w) -> p c w", c=4),
                        )

            import contextlib
            loop_ctx = tc.For_i(0, loop_n, 1) if loop_n else contextlib.nullcontext()
            with loop_ctx:
                _run_iter()

    nc.compile()
    return nc


def _get_module():
    if "nc" not in _CACHE:
        _CACHE["nc"] = _build_module()
    return _CACHE["nc"]


def make_in_maps(x, qkv_w, proj_w):
    xf = np.asarray(x, dtype=np.float32).reshape(B, C, N)
    wq = np.ascontiguousarray(np.asarray(qkv_w).T).astype(np.float16)
    wpj = np.ascontiguousarray(np.asarray(proj_w).T).astype(np.float16)
    in_maps = []
    for i in range(NCORES):
        b, h = divmod(i, 2)
        # roll so this core's query half sits at cols 0:NQ; attention is
        # permutation-invariant over key positions so this is exact.
        xkc = np.ascontiguousarray(np.roll(xf[b], -h * NQ, axis=1)).astype(np.float16)
        in_maps.append({"xk": xkc, "wqkv": wq, "wproj": wpj})
    return in_maps


def gather_out(results):
    out = np.empty((B, C, N), np.float32)
    for i in range(NCORES):
        b, h = divmod(i, 2)
        out[b][:, h * NQ : (h + 1) * NQ] = results[i]["y"].astype(np.float32)
    return out.reshape(B, C, HH, WW)


def kernel(x, qkv_w, proj_w):
    from concourse import bass_utils

    nc = _get_module()
    in_maps = make_in_maps(x, qkv_w, proj_w)
    res = bass_utils.run_bass_kernel_spmd(
        nc, in_maps, core_ids=list(range(NCORES)), trace=False
    )
    return gather_out(res.results)
